# revision 7
# baseline (speedup 1.0000x reference)
"""Trainium2 Bass kernel for fused LN + QKV + partial-RoPE attention + out-proj.

Sharding: 8 cores = 4 batches x 2 head-groups (4 heads each).
Core c: batch = c % 4, heads = [4*(c//4) .. 4*(c//4)+3].
Each core returns a partial y^T [DIM, N]; host sums the two head-group
partials per batch and adds b_out.

Device design (per core):
  - LayerNorm in token-major tiles [128n, 128d]; stats batched [128, 16].
  - xn transposed via PE into xnT [DIM, N] (bf16).
  - Per head: qT/rotqT/kT/rotkT = W @ xnT (rotation folded into host
    precomputed weights); RoPE = q*cos + rot*sin on DVE; v computed
    token-major directly.
  - Attention with TRANSPOSED scores: scoresT[j, q] = k_jb^T-stationary
    matmuls, so probs come out with j on partitions and feed the AV matmul
    with no transposes. exp on ACT (scale folds 1/sqrt(d)); row sums via
    all-ones stationary matmul (output is the broadcast R); 1/R via
    ACT exp(-ln(R)) (same table set as Exp); normalize folded into the
    PSUM->SBUF copy of the AV output on DVE.
"""

import numpy as np
import ml_dtypes
from contextlib import ExitStack

import concourse.bass as bass
import concourse.tile as tile
from concourse import bacc
from concourse import mybir
from concourse.bass import ts
from concourse.bass_utils import run_bass_kernel_spmd

B, N, DIM = 4, 2048, 128
HEADS, HEAD = 8, 128
INNER = HEADS * HEAD
HPC = 4            # heads per core
NT = N // 128      # 16 token tiles
EPS = 1e-5
SCALE = HEAD ** -0.5

F32 = mybir.dt.float32
BF16 = mybir.dt.bfloat16
AF = mybir.ActivationFunctionType
ALU = mybir.AluOpType
AX = mybir.AxisListType

BF16_NP = ml_dtypes.bfloat16

_CACHE = {}


def _build_nc():
    nc = bacc.Bacc()
    x_d = nc.declare_dram_parameter("x", [N, DIM], F32, isOutput=False)
    wqkv_d = nc.declare_dram_parameter("wqkv", [128, HPC * 5 * 128], BF16, isOutput=False)
    wo_d = nc.declare_dram_parameter("wo", [128, HPC * 128], BF16, isOutput=False)
    cos_d = nc.declare_dram_parameter("cost", [128, N], F32, isOutput=False)
    sin_d = nc.declare_dram_parameter("sint", [128, N], F32, isOutput=False)
    ident_d = nc.declare_dram_parameter("ident", [128, 128], BF16, isOutput=False)
    ones_d = nc.declare_dram_parameter("ones", [128, 128], BF16, isOutput=False)
    yt_d = nc.declare_dram_parameter("yt", [128, N], F32, isOutput=True)

    with ExitStack() as ctx:
        tc = ctx.enter_context(tile.TileContext(nc))
        const = ctx.enter_context(tc.tile_pool(name="const", bufs=1))
        sb = ctx.enter_context(tc.tile_pool(name="sb", bufs=1))
        rope_p = ctx.enter_context(tc.tile_pool(name="rope", bufs=2))
        qk_p = ctx.enter_context(tc.tile_pool(name="qk", bufs=2))
        exp_p = ctx.enter_context(tc.tile_pool(name="exps", bufs=17))
        on_p = ctx.enter_context(tc.tile_pool(name="onorm", bufs=HPC))
        ps_s = ctx.enter_context(tc.tile_pool(name="ps_s", bufs=1, space="PSUM"))
        ps_a = ctx.enter_context(tc.tile_pool(name="ps_a", bufs=1, space="PSUM"))

        # ---------------- constants ----------------
        cos_t = const.tile([128, N], F32, tag="cos")
        nc.sync.dma_start(out=cos_t, in_=cos_d[:, :])
        sin_t = const.tile([128, N], F32, tag="sin")
        nc.sync.dma_start(out=sin_t, in_=sin_d[:, :])
        wqkv_t = const.tile([128, HPC * 5 * 128], BF16, tag="wqkv")
        nc.sync.dma_start(out=wqkv_t, in_=wqkv_d[:, :])
        wo_t = const.tile([128, HPC * 128], BF16, tag="wo")
        nc.sync.dma_start(out=wo_t, in_=wo_d[:, :])
        ident_t = const.tile([128, 128], BF16, tag="ident")
        nc.sync.dma_start(out=ident_t, in_=ident_d[:, :])
        ones_t = const.tile([128, 128], BF16, tag="ones")
        nc.sync.dma_start(out=ones_t, in_=ones_d[:, :])

        def W(h, i):
            return wqkv_t[:, ts(h * 5 + i, 128)]

        # ---------------- LayerNorm ----------------
        xt_p = ctx.enter_context(tc.tile_pool(name="xt", bufs=NT))
        xts = []
        for t in range(NT):
            xt = xt_p.tile([128, 128], F32, tag="xt")
            nc.sync.dma_start(out=xt, in_=x_d[t * 128:(t + 1) * 128, :])
            xts.append(xt)

        st_sum = const.tile([128, NT], F32, tag="st_sum")
        st_sq = const.tile([128, NT], F32, tag="st_sq")
        sq_p = ctx.enter_context(tc.tile_pool(name="sq", bufs=3))
        for t in range(NT):
            nc.vector.tensor_reduce(
                out=st_sum[:, t:t + 1], in_=xts[t], axis=AX.X, op=ALU.add)
            sq = sq_p.tile([128, 128], F32, tag="sq")
            nc.scalar.activation(out=sq, in_=xts[t], func=AF.Square)
            nc.vector.tensor_reduce(
                out=st_sq[:, t:t + 1], in_=sq, axis=AX.X, op=ALU.add)

        mean = const.tile([128, NT], F32, tag="mean")
        nc.vector.tensor_scalar_mul(mean, st_sum, 1.0 / DIM)
        msq = const.tile([128, NT], F32, tag="msq")
        nc.scalar.activation(out=msq, in_=mean, func=AF.Square)
        var = const.tile([128, NT], F32, tag="var")
        nc.vector.scalar_tensor_tensor(
            out=var, in0=st_sq, scalar=1.0 / DIM, in1=msq,
            op0=ALU.mult, op1=ALU.subtract)
        epsb = const.tile([128, 1], F32, tag="epsb")
        nc.vector.memset(epsb, EPS)
        lnv = const.tile([128, NT], F32, tag="lnv")
        nc.scalar.activation(out=lnv, in_=var, func=AF.Ln, bias=epsb)
        istd = const.tile([128, NT], F32, tag="istd")
        nc.scalar.activation(out=istd, in_=lnv, func=AF.Exp, scale=-0.5)
        nbias = const.tile([128, NT], F32, tag="nbias")
        nc.vector.scalar_tensor_tensor(
            out=nbias, in0=mean, scalar=-1.0, in1=istd,
            op0=ALU.mult, op1=ALU.mult)

        xn = const.tile([128, N], BF16, tag="xn")
        for t in range(NT):
            nc.scalar.activation(
                out=xn[:, ts(t, 128)], in_=xts[t], func=AF.Identity,
                scale=istd[:, t:t + 1], bias=nbias[:, t:t + 1])

        # transpose xn -> xnT [DIM, N]
        xnT_ps = ps_a.tile([128, N], BF16, tag="a")
        for t in range(NT):
            nc.tensor.transpose(
                out=xnT_ps[:, ts(t, 128)], in_=xn[:, ts(t, 128)], identity=ident_t)
        xnT = const.tile([128, N], BF16, tag="xnT")
        nc.scalar.copy(out=xnT, in_=xnT_ps)

        # ---------------- per-head attention ----------------
        onorm = []
        for h in range(HPC):
            # Q projection + rope
            q_ps = ps_s.tile([128, N], F32, tag="s")
            for c in range(4):
                nc.tensor.matmul(out=q_ps[:, ts(c, 512)], lhsT=W(h, 0),
                                 rhs=xnT[:, ts(c, 512)], start=True, stop=True)
            qr_ps = ps_a.tile([128, N], F32, tag="a")
            for c in range(4):
                nc.tensor.matmul(out=qr_ps[:, ts(c, 512)], lhsT=W(h, 1),
                                 rhs=xnT[:, ts(c, 512)], start=True, stop=True)
            t1 = rope_p.tile([128, N], F32, tag="rope1")
            nc.vector.tensor_mul(t1, q_ps, cos_t)
            t2 = rope_p.tile([128, N], F32, tag="rope2")
            nc.vector.tensor_mul(t2, qr_ps, sin_t)
            qh = qk_p.tile([128, N], BF16, tag="qrope")
            nc.vector.tensor_add(qh, t1, t2)

            # K projection + rope
            k_ps = ps_s.tile([128, N], F32, tag="s")
            for c in range(4):
                nc.tensor.matmul(out=k_ps[:, ts(c, 512)], lhsT=W(h, 2),
                                 rhs=xnT[:, ts(c, 512)], start=True, stop=True)
            kr_ps = ps_a.tile([128, N], F32, tag="a")
            for c in range(4):
                nc.tensor.matmul(out=kr_ps[:, ts(c, 512)], lhsT=W(h, 3),
                                 rhs=xnT[:, ts(c, 512)], start=True, stop=True)
            t1k = rope_p.tile([128, N], F32, tag="rope1")
            nc.vector.tensor_mul(t1k, k_ps, cos_t)
            t2k = rope_p.tile([128, N], F32, tag="rope2")
            nc.vector.tensor_mul(t2k, kr_ps, sin_t)
            kh = qk_p.tile([128, N], BF16, tag="krope")
            nc.vector.tensor_add(kh, t1k, t2k)

            # V token-major: v[nchunk, d] = xnT[:, chunk].T @ Wv^T
            v_ps = ps_s.tile([128, N], F32, tag="s")
            for c in range(NT):
                nc.tensor.matmul(out=v_ps[:, ts(c, 128)], lhsT=xnT[:, ts(c, 128)],
                                 rhs=W(h, 4), start=True, stop=True)
            vh = qk_p.tile([128, N], BF16, tag="vsb")
            nc.vector.tensor_copy(vh, v_ps)

            # phase A: scoresT -> exp -> R accumulation
            exps = []
            R_ps = ps_a.tile([128, N], F32, tag="a")
            for jb in range(NT):
                s_ps = ps_s.tile([128, N], F32, tag="s")
                for c in range(4):
                    nc.tensor.matmul(out=s_ps[:, ts(c, 512)], lhsT=kh[:, ts(jb, 128)],
                                     rhs=qh[:, ts(c, 512)], start=True, stop=True)
                e = exp_p.tile([128, N], BF16, tag="expT")
                nc.scalar.activation(out=e, in_=s_ps, func=AF.Exp, scale=SCALE)
                exps.append(e)
                for c in range(4):
                    nc.tensor.matmul(out=R_ps[:, ts(c, 512)], lhsT=ones_t,
                                     rhs=e[:, ts(c, 512)],
                                     start=(jb == 0), stop=(jb == NT - 1),
                                     skip_group_check=True)
            lnR = rope_p.tile([128, N], F32, tag="rope1")
            nc.scalar.activation(out=lnR, in_=R_ps, func=AF.Ln)
            rinv = rope_p.tile([128, N], F32, tag="rope2")
            nc.scalar.activation(out=rinv, in_=lnR, func=AF.Exp, scale=-1.0)

            # phase B: AV accumulation, then fold in 1/R on the way to SBUF
            o_ps = ps_a.tile([128, N], F32, tag="a")
            for jb in range(NT):
                for c in range(4):
                    nc.tensor.matmul(out=o_ps[:, ts(c, 512)], lhsT=vh[:, ts(jb, 128)],
                                     rhs=exps[jb][:, ts(c, 512)],
                                     start=(jb == 0), stop=(jb == NT - 1),
                                     skip_group_check=True)
            oh = on_p.tile([128, N], BF16, tag="onorm")
            nc.vector.tensor_mul(oh, o_ps, rinv)
            onorm.append(oh)

        # ---------------- output projection ----------------
        y_ps = ps_s.tile([128, N], F32, tag="s")
        for h in range(HPC):
            for c in range(4):
                nc.tensor.matmul(out=y_ps[:, ts(c, 512)], lhsT=wo_t[:, ts(h, 128)],
                                 rhs=onorm[h][:, ts(c, 512)],
                                 start=(h == 0), stop=(h == HPC - 1),
                                 skip_group_check=True)
        y_sb = sb.tile([128, N], F32, tag="ysb")
        nc.vector.tensor_copy(y_sb, y_ps)
        nc.sync.dma_start(out=yt_d[:, :], in_=y_sb)

    nc.finalize()
    return nc


def _rope_tables():
    """cos/sin tables in [d, n] layout; token N-1 unrotated; sin sign-folded."""
    inv_freq = 1.0 / (10000.0 ** (np.arange(0, HEAD, 2, dtype=np.float64) / HEAD))
    pos = np.arange(N, dtype=np.float64)
    ang = pos[None, :] * np.repeat(inv_freq, 2)[:, None]        # [d, n]
    cos_t = np.cos(ang)
    sin_t = np.sin(ang)
    sign = np.where(np.arange(HEAD) % 2 == 0, -1.0, 1.0)[:, None]
    sin_t = sin_t * sign
    cos_t[:, N - 1] = 1.0
    sin_t[:, N - 1] = 0.0
    return cos_t.astype(np.float32), sin_t.astype(np.float32)


def _prep_core_inputs(x, ln_gamma, ln_beta, w_qkv, w_out):
    """Build the 8 per-core input maps (host-side layout/packing)."""
    cos_t, sin_t = _rope_tables()
    ident = np.eye(128, dtype=np.float32)
    ones = np.ones((128, 128), dtype=np.float32)

    swap = np.arange(HEAD) ^ 1                                  # pair swap perm
    in_maps = []
    for c in range(8):
        b = c % 4
        g = c // 4
        wq_blocks = []
        for i in range(HPC):
            h = g * HPC + i
            Wq = w_qkv[h * HEAD:(h + 1) * HEAD, :] * ln_gamma[None, :]
            Wk = w_qkv[INNER + h * HEAD:INNER + (h + 1) * HEAD, :] * ln_gamma[None, :]
            Wv = w_qkv[2 * INNER + h * HEAD:2 * INNER + (h + 1) * HEAD, :] * ln_gamma[None, :]
            wq_blocks += [Wq.T, Wq[swap, :].T, Wk.T, Wk[swap, :].T, Wv.T]
        wqkv_packed = np.concatenate(wq_blocks, axis=1)          # [128, HPC*5*128]
        wo_packed = np.concatenate(
            [w_out[:, (g * HPC + i) * HEAD:(g * HPC + i + 1) * HEAD].T
             for i in range(HPC)], axis=1)                       # [d, HPC*128] -> lhsT per head
        in_maps.append({
            "x": np.ascontiguousarray(x[b], dtype=np.float32),
            "wqkv": wqkv_packed.astype(BF16_NP),
            "wo": wo_packed.astype(BF16_NP),
            "cost": cos_t,
            "sint": sin_t,
            "ident": ident.astype(BF16_NP),
            "ones": ones.astype(BF16_NP),
        })
    return in_maps


def kernel(x, ln_gamma, ln_beta, w_qkv, w_out, b_out):
    x = np.asarray(x, dtype=np.float32)
    ln_gamma = np.asarray(ln_gamma, dtype=np.float32)
    ln_beta = np.asarray(ln_beta, dtype=np.float32)
    w_qkv = np.asarray(w_qkv, dtype=np.float32)
    w_out = np.asarray(w_out, dtype=np.float32)
    b_out = np.asarray(b_out, dtype=np.float32)
    assert np.allclose(ln_beta, 0.0), "beta folding not implemented"

    if "nc" not in _CACHE:
        _CACHE["nc"] = _build_nc()
    nc = _CACHE["nc"]

    in_maps = _prep_core_inputs(x, ln_gamma, ln_beta, w_qkv, w_out)
    res = run_bass_kernel_spmd(nc, in_maps, list(range(8)))
    _CACHE["last_results"] = res

    out = np.empty((B, N, DIM), dtype=np.float32)
    for b in range(B):
        y0 = np.asarray(res.results[b]["yt"], dtype=np.float32)
        y1 = np.asarray(res.results[b + 4]["yt"], dtype=np.float32)
        out[b] = (y0 + y1).T + b_out[None, :]
    return out


# revision 9
# speedup vs baseline: 1.1643x; 1.1643x over previous
"""Trainium2 Bass kernel for fused LN + QKV + partial-RoPE attention + out-proj.

Sharding: 8 cores = 4 batches x 2 head-groups (4 heads each).
Core c: batch = c % 4, heads = [4*(c//4) .. 4*(c//4)+3].
Each core returns a partial y^T [DIM, N]; host sums the two head-group
partials per batch and adds b_out.

Device design (per core):
  - LayerNorm in token-major tiles [128n, 128d]; stats batched [128, 16].
  - xn transposed via PE into xnT [DIM, N] (bf16).
  - Per head: qT/rotqT/kT/rotkT = W @ xnT (rotation folded into host
    precomputed weights); RoPE = q*cos + rot*sin on DVE; v computed
    token-major directly.
  - Attention with TRANSPOSED scores: scoresT[j, q] = k_jb^T-stationary
    matmuls, so probs come out with j on partitions and feed the AV matmul
    with no transposes. exp on ACT (scale folds 1/sqrt(d)); row sums via
    all-ones stationary matmul (output is the broadcast R); 1/R via
    ACT exp(-ln(R)) (same table set as Exp); normalize folded into the
    PSUM->SBUF copy of the AV output on DVE.
"""

import numpy as np
import ml_dtypes
from contextlib import ExitStack

import concourse.bass as bass
import concourse.tile as tile
from concourse import bacc
from concourse import mybir
from concourse.bass import ts
from concourse.bass_utils import run_bass_kernel_spmd

B, N, DIM = 4, 2048, 128
HEADS, HEAD = 8, 128
INNER = HEADS * HEAD
HPC = 4            # heads per core
NT = N // 128      # 16 token tiles
EPS = 1e-5
SCALE = HEAD ** -0.5

F32 = mybir.dt.float32
BF16 = mybir.dt.bfloat16
AF = mybir.ActivationFunctionType
ALU = mybir.AluOpType
AX = mybir.AxisListType

BF16_NP = ml_dtypes.bfloat16

_CACHE = {}


def _build_nc():
    nc = bacc.Bacc()
    x_d = nc.declare_dram_parameter("x", [N, DIM], F32, isOutput=False)
    wqkv_d = nc.declare_dram_parameter("wqkv", [128, HPC * 5 * 128], BF16, isOutput=False)
    wo_d = nc.declare_dram_parameter("wo", [128, HPC * 128], BF16, isOutput=False)
    cos_d = nc.declare_dram_parameter("cost", [128, N], F32, isOutput=False)
    sin_d = nc.declare_dram_parameter("sint", [128, N], F32, isOutput=False)
    ident_d = nc.declare_dram_parameter("ident", [128, 128], BF16, isOutput=False)
    ones_d = nc.declare_dram_parameter("ones", [128, 128], BF16, isOutput=False)
    yt_d = nc.declare_dram_parameter("yt", [128, N], F32, isOutput=True)

    with ExitStack() as ctx:
        tc = ctx.enter_context(tile.TileContext(nc))
        const = ctx.enter_context(tc.tile_pool(name="const", bufs=1))
        sb = ctx.enter_context(tc.tile_pool(name="sb", bufs=1))
        rope_p = ctx.enter_context(tc.tile_pool(name="rope", bufs=2))
        qk_p = ctx.enter_context(tc.tile_pool(name="qk", bufs=2))
        exp_p = ctx.enter_context(tc.tile_pool(name="exps", bufs=17))
        on_p = ctx.enter_context(tc.tile_pool(name="onorm", bufs=HPC))
        ps_s = ctx.enter_context(tc.tile_pool(name="ps_s", bufs=1, space="PSUM"))
        ps_a = ctx.enter_context(tc.tile_pool(name="ps_a", bufs=1, space="PSUM"))

        # ---------------- constants ----------------
        cos_t = const.tile([128, N], F32, tag="cos")
        nc.sync.dma_start(out=cos_t, in_=cos_d[:, :])
        sin_t = const.tile([128, N], F32, tag="sin")
        nc.sync.dma_start(out=sin_t, in_=sin_d[:, :])
        wqkv_t = const.tile([128, HPC * 5 * 128], BF16, tag="wqkv")
        nc.sync.dma_start(out=wqkv_t, in_=wqkv_d[:, :])
        wo_t = const.tile([128, HPC * 128], BF16, tag="wo")
        nc.sync.dma_start(out=wo_t, in_=wo_d[:, :])
        ident_t = const.tile([128, 128], BF16, tag="ident")
        nc.sync.dma_start(out=ident_t, in_=ident_d[:, :])
        ones_t = const.tile([128, 128], BF16, tag="ones")
        nc.sync.dma_start(out=ones_t, in_=ones_d[:, :])

        def W(h, i):
            return wqkv_t[:, ts(h * 5 + i, 128)]

        # ---------------- LayerNorm ----------------
        xt_p = ctx.enter_context(tc.tile_pool(name="xt", bufs=NT))
        xts = []
        for t in range(NT):
            xt = xt_p.tile([128, 128], F32, tag="xt")
            nc.sync.dma_start(out=xt, in_=x_d[t * 128:(t + 1) * 128, :])
            xts.append(xt)

        st_sum = const.tile([128, NT], F32, tag="st_sum")
        st_sq = const.tile([128, NT], F32, tag="st_sq")
        sq_p = ctx.enter_context(tc.tile_pool(name="sq", bufs=3))
        for t in range(NT):
            nc.vector.tensor_reduce(
                out=st_sum[:, t:t + 1], in_=xts[t], axis=AX.X, op=ALU.add)
            sq = sq_p.tile([128, 128], F32, tag="sq")
            nc.scalar.activation(out=sq, in_=xts[t], func=AF.Square)
            nc.vector.tensor_reduce(
                out=st_sq[:, t:t + 1], in_=sq, axis=AX.X, op=ALU.add)

        mean = const.tile([128, NT], F32, tag="mean")
        nc.vector.tensor_scalar_mul(mean, st_sum, 1.0 / DIM)
        msq = const.tile([128, NT], F32, tag="msq")
        nc.scalar.activation(out=msq, in_=mean, func=AF.Square)
        var = const.tile([128, NT], F32, tag="var")
        nc.vector.scalar_tensor_tensor(
            out=var, in0=st_sq, scalar=1.0 / DIM, in1=msq,
            op0=ALU.mult, op1=ALU.subtract)
        epsb = const.tile([128, 1], F32, tag="epsb")
        nc.vector.memset(epsb, EPS)
        lnv = const.tile([128, NT], F32, tag="lnv")
        nc.scalar.activation(out=lnv, in_=var, func=AF.Ln, bias=epsb)
        istd = const.tile([128, NT], F32, tag="istd")
        nc.scalar.activation(out=istd, in_=lnv, func=AF.Exp, scale=-0.5)
        nbias = const.tile([128, NT], F32, tag="nbias")
        nc.vector.scalar_tensor_tensor(
            out=nbias, in0=mean, scalar=-1.0, in1=istd,
            op0=ALU.mult, op1=ALU.mult)

        xn = const.tile([128, N], BF16, tag="xn")
        for t in range(NT):
            nc.scalar.activation(
                out=xn[:, ts(t, 128)], in_=xts[t], func=AF.Identity,
                scale=istd[:, t:t + 1], bias=nbias[:, t:t + 1])

        # transpose xn -> xnT [DIM, N]
        xnT_ps = ps_a.tile([128, N], BF16, tag="a")
        for t in range(NT):
            nc.tensor.transpose(
                out=xnT_ps[:, ts(t, 128)], in_=xn[:, ts(t, 128)], identity=ident_t)
        xnT = const.tile([128, N], BF16, tag="xnT")
        nc.scalar.copy(out=xnT, in_=xnT_ps)

        # ---------------- per-head attention ----------------
        onorm = []
        for h in range(HPC):
            # Q projection + rope
            q_ps = ps_s.tile([128, N], F32, tag="s")
            for c in range(4):
                nc.tensor.matmul(out=q_ps[:, ts(c, 512)], lhsT=W(h, 0),
                                 rhs=xnT[:, ts(c, 512)], start=True, stop=True)
            qr_ps = ps_a.tile([128, N], F32, tag="a")
            for c in range(4):
                nc.tensor.matmul(out=qr_ps[:, ts(c, 512)], lhsT=W(h, 1),
                                 rhs=xnT[:, ts(c, 512)], start=True, stop=True)
            t1 = rope_p.tile([128, N], F32, tag="rope1")
            nc.vector.tensor_mul(t1, q_ps, cos_t)
            t2 = rope_p.tile([128, N], F32, tag="rope2")
            nc.vector.tensor_mul(t2, qr_ps, sin_t)
            qh = qk_p.tile([128, N], BF16, tag="qrope")
            nc.vector.tensor_add(qh, t1, t2)

            # K projection + rope
            k_ps = ps_s.tile([128, N], F32, tag="s")
            for c in range(4):
                nc.tensor.matmul(out=k_ps[:, ts(c, 512)], lhsT=W(h, 2),
                                 rhs=xnT[:, ts(c, 512)], start=True, stop=True)
            kr_ps = ps_a.tile([128, N], F32, tag="a")
            for c in range(4):
                nc.tensor.matmul(out=kr_ps[:, ts(c, 512)], lhsT=W(h, 3),
                                 rhs=xnT[:, ts(c, 512)], start=True, stop=True)
            t1k = rope_p.tile([128, N], F32, tag="rope1")
            nc.vector.tensor_mul(t1k, k_ps, cos_t)
            t2k = rope_p.tile([128, N], F32, tag="rope2")
            nc.vector.tensor_mul(t2k, kr_ps, sin_t)
            kh = qk_p.tile([128, N], BF16, tag="krope")
            nc.vector.tensor_add(kh, t1k, t2k)

            # V token-major: v[nchunk, d] = xnT[:, chunk].T @ Wv^T
            v_ps = ps_s.tile([128, N], F32, tag="s")
            for c in range(NT):
                nc.tensor.matmul(out=v_ps[:, ts(c, 128)], lhsT=xnT[:, ts(c, 128)],
                                 rhs=W(h, 4), start=True, stop=True)
            vh = qk_p.tile([128, N], BF16, tag="vsb")
            nc.vector.tensor_copy(vh, v_ps)

            # phase A: scoresT -> exp -> R accumulation
            exps = []
            R_ps = ps_a.tile([128, N], F32, tag="a")
            for jb in range(NT):
                s_ps = ps_s.tile([128, N], F32, tag="s")
                for c in range(4):
                    nc.tensor.matmul(out=s_ps[:, ts(c, 512)], lhsT=kh[:, ts(jb, 128)],
                                     rhs=qh[:, ts(c, 512)], start=True, stop=True)
                e = exp_p.tile([128, N], BF16, tag="expT")
                nc.scalar.activation(out=e, in_=s_ps, func=AF.Exp, scale=SCALE)
                exps.append(e)
                for c in range(4):
                    nc.tensor.matmul(out=R_ps[:, ts(c, 512)], lhsT=ones_t,
                                     rhs=e[:, ts(c, 512)],
                                     start=(jb == 0), stop=(jb == NT - 1),
                                     skip_group_check=True)
            lnR = rope_p.tile([128, N], F32, tag="rope1")
            nc.scalar.activation(out=lnR, in_=R_ps, func=AF.Ln)
            rinv = rope_p.tile([128, N], F32, tag="rope2")
            nc.scalar.activation(out=rinv, in_=lnR, func=AF.Exp, scale=-1.0)

            # phase B: AV accumulation, then fold in 1/R on the way to SBUF
            o_ps = ps_a.tile([128, N], F32, tag="a")
            for jb in range(NT):
                for c in range(4):
                    nc.tensor.matmul(out=o_ps[:, ts(c, 512)], lhsT=vh[:, ts(jb, 128)],
                                     rhs=exps[jb][:, ts(c, 512)],
                                     start=(jb == 0), stop=(jb == NT - 1),
                                     skip_group_check=True)
            oh = on_p.tile([128, N], BF16, tag="onorm")
            nc.vector.tensor_mul(oh, o_ps, rinv)
            onorm.append(oh)

        # ---------------- output projection ----------------
        y_ps = ps_s.tile([128, N], F32, tag="s")
        for h in range(HPC):
            for c in range(4):
                nc.tensor.matmul(out=y_ps[:, ts(c, 512)], lhsT=wo_t[:, ts(h, 128)],
                                 rhs=onorm[h][:, ts(c, 512)],
                                 start=(h == 0), stop=(h == HPC - 1),
                                 skip_group_check=True)
        y_sb = sb.tile([128, N], F32, tag="ysb")
        nc.vector.tensor_copy(y_sb, y_ps)
        nc.sync.dma_start(out=yt_d[:, :], in_=y_sb)

    nc.finalize()
    return nc



def _make_runner(nc, n_cores=8):
    """Cached jitted multi-core executor (mirrors bass2jax.run_bass_via_pjrt,
    minus output-donation so it can be called repeatedly for timing)."""
    import jax
    import jax.numpy as jnp
    from jax.sharding import Mesh, PartitionSpec
    from jax.experimental.shard_map import shard_map
    from concourse import bass2jax, mybir as mb
    bass2jax.install_neuronx_cc_hook()

    partition_name = nc.partition_id_tensor.name if nc.partition_id_tensor else None
    in_names, out_names, out_avals, zero_outs = [], [], [], []
    for alloc in nc.m.functions[0].allocations:
        if not isinstance(alloc, mb.MemoryLocationSet):
            continue
        name = alloc.memorylocations[0].name
        if alloc.kind == "ExternalInput":
            if name != partition_name:
                in_names.append(name)
        elif alloc.kind == "ExternalOutput":
            out_names.append(name)
            shape = tuple(alloc.tensor_shape)
            dtype = mb.dt.np(alloc.dtype)
            out_avals.append(jax.core.ShapedArray(shape, dtype))
            zero_outs.append(np.zeros(shape, dtype))
    n_params = len(in_names)
    all_in_names = list(in_names) + list(out_names)
    if partition_name is not None:
        all_in_names.append(partition_name)

    def _body(*args):
        operands = list(args)
        if partition_name is not None:
            operands.append(bass2jax.partition_id_tensor())
        outs = bass2jax._bass_exec_p.bind(
            *operands,
            out_avals=tuple(out_avals),
            in_names=tuple(all_in_names),
            out_names=tuple(out_names),
            lowering_input_output_aliases=(),
            sim_require_finite=True,
            sim_require_nnan=True,
            nc=nc,
        )
        return tuple(outs)

    devices = jax.devices()[:n_cores]
    mesh = Mesh(np.asarray(devices), ("core",))
    in_specs = (PartitionSpec("core"),) * (n_params + len(out_names))
    out_specs = (PartitionSpec("core"),) * len(out_names)
    donate = tuple(range(n_params, n_params + len(out_names)))
    sharded = jax.jit(shard_map(_body, mesh=mesh, in_specs=in_specs,
                                out_specs=out_specs, check_rep=False),
                      donate_argnums=donate, keep_unused=True)

    def run(in_maps):
        concat_in = [np.concatenate([np.asarray(in_maps[c][k]) for c in range(n_cores)], axis=0)
                     for k in in_names]
        concat_zero = [np.concatenate([z] * n_cores, axis=0) for z in zero_outs]
        outs = sharded(*concat_in, *concat_zero)
        outs = [np.asarray(o) for o in outs]
        res = []
        for c in range(n_cores):
            d = {}
            for i, name in enumerate(out_names):
                per = outs[i].shape[0] // n_cores
                d[name] = outs[i][c * per:(c + 1) * per]
            res.append(d)
        return res, sharded, (in_names, zero_outs)

    return run


def _rope_tables():
    """cos/sin tables in [d, n] layout; token N-1 unrotated; sin sign-folded."""
    inv_freq = 1.0 / (10000.0 ** (np.arange(0, HEAD, 2, dtype=np.float64) / HEAD))
    pos = np.arange(N, dtype=np.float64)
    ang = pos[None, :] * np.repeat(inv_freq, 2)[:, None]        # [d, n]
    cos_t = np.cos(ang)
    sin_t = np.sin(ang)
    sign = np.where(np.arange(HEAD) % 2 == 0, -1.0, 1.0)[:, None]
    sin_t = sin_t * sign
    cos_t[:, N - 1] = 1.0
    sin_t[:, N - 1] = 0.0
    return cos_t.astype(np.float32), sin_t.astype(np.float32)


def _prep_core_inputs(x, ln_gamma, ln_beta, w_qkv, w_out):
    """Build the 8 per-core input maps (host-side layout/packing)."""
    cos_t, sin_t = _rope_tables()
    ident = np.eye(128, dtype=np.float32)
    ones = np.ones((128, 128), dtype=np.float32)

    swap = np.arange(HEAD) ^ 1                                  # pair swap perm
    in_maps = []
    for c in range(8):
        b = c % 4
        g = c // 4
        wq_blocks = []
        for i in range(HPC):
            h = g * HPC + i
            Wq = w_qkv[h * HEAD:(h + 1) * HEAD, :] * ln_gamma[None, :]
            Wk = w_qkv[INNER + h * HEAD:INNER + (h + 1) * HEAD, :] * ln_gamma[None, :]
            Wv = w_qkv[2 * INNER + h * HEAD:2 * INNER + (h + 1) * HEAD, :] * ln_gamma[None, :]
            wq_blocks += [Wq.T, Wq[swap, :].T, Wk.T, Wk[swap, :].T, Wv.T]
        wqkv_packed = np.concatenate(wq_blocks, axis=1)          # [128, HPC*5*128]
        wo_packed = np.concatenate(
            [w_out[:, (g * HPC + i) * HEAD:(g * HPC + i + 1) * HEAD].T
             for i in range(HPC)], axis=1)                       # [d, HPC*128] -> lhsT per head
        in_maps.append({
            "x": np.ascontiguousarray(x[b], dtype=np.float32),
            "wqkv": wqkv_packed.astype(BF16_NP),
            "wo": wo_packed.astype(BF16_NP),
            "cost": cos_t,
            "sint": sin_t,
            "ident": ident.astype(BF16_NP),
            "ones": ones.astype(BF16_NP),
        })
    return in_maps


def kernel(x, ln_gamma, ln_beta, w_qkv, w_out, b_out):
    x = np.asarray(x, dtype=np.float32)
    ln_gamma = np.asarray(ln_gamma, dtype=np.float32)
    ln_beta = np.asarray(ln_beta, dtype=np.float32)
    w_qkv = np.asarray(w_qkv, dtype=np.float32)
    w_out = np.asarray(w_out, dtype=np.float32)
    b_out = np.asarray(b_out, dtype=np.float32)
    assert np.allclose(ln_beta, 0.0), "beta folding not implemented"

    if "nc" not in _CACHE:
        _CACHE["nc"] = _build_nc()
    nc = _CACHE["nc"]

    in_maps = _prep_core_inputs(x, ln_gamma, ln_beta, w_qkv, w_out)
    if "runner" not in _CACHE:
        _CACHE["runner"] = _make_runner(nc)
    results, _, _ = _CACHE["runner"](in_maps)
    _CACHE["last_in_maps"] = in_maps

    out = np.empty((B, N, DIM), dtype=np.float32)
    for b in range(B):
        y0 = np.asarray(results[b]["yt"], dtype=np.float32)
        y1 = np.asarray(results[b + 4]["yt"], dtype=np.float32)
        out[b] = (y0 + y1).T + b_out[None, :]
    return out
